# revision 1
# baseline (speedup 1.0000x reference)
"""Causal self-attention (B=4, T=2048, C=1024, 16 heads x 64) on 8 TRN2 NeuronCores.

Sharding: tensor-parallel over heads. Core c owns heads {2c, 2c+1}:
  - w_attn column slices -> per-core QKV in transposed layout (dims on
    partitions, tokens on free dim),
  - attention in S^T form: S^T[k,q] = matmul(lhsT=kT, rhs=qT_headzeroed),
    softmax denominator via ones-columns appended to V, PV consumes exp(S^T)
    directly (no transposes anywhere in the attention inner loop),
  - partial output projection in transposed layout (out dims on partitions,
    so b_proj is a per-partition bias),
  - host sums the 8 partial projections (the TP all-reduce).

All matmuls run as float32r (TF32-like, ~1.6e-4 per-matmul rel err, 4x the
fp32 matmul rate on the PE).
"""

import sys
import numpy as np

sys.path.insert(0, "/opt/trn_rl_repo")

B, T, C = 4, 2048, 1024
NH, HD = 16, 64
NCORES = 8
TOK = B * T                 # 8192 tokens
NCH = TOK // 512            # 16 token chunks of 512
CHB = T // 512              # 4 chunks per batch
NKB_B = T // 128            # 16 k-blocks per batch
SCALE = 1.0 / 8.0

_CACHE = {}


def _build_program():
    import concourse.tile as tile
    from concourse import bacc, mybir
    from concourse.masks import make_identity

    f32 = mybir.dt.float32
    f32r = mybir.dt.float32r

    nc = bacc.Bacc("TRN2", target_bir_lowering=False, debug=False,
                   num_devices=NCORES)

    xT = nc.dram_tensor("xT", [C, TOK], f32r, kind="ExternalInput").ap()
    wqkv = nc.dram_tensor("wqkv", [C, 384], f32r, kind="ExternalInput").ap()
    battn = nc.dram_tensor("battn", [128, 3], f32, kind="ExternalInput").ap()
    wproj = nc.dram_tensor("wproj", [128, C], f32r, kind="ExternalInput").ap()
    bproj = nc.dram_tensor("bproj", [128, 8], f32, kind="ExternalInput").ap()
    outT = nc.dram_tensor("outT", [C, TOK], f32, kind="ExternalOutput").ap()

    with tile.TileContext(nc) as tc:
        with tc.tile_pool(name="const", bufs=1) as const, \
             tc.tile_pool(name="resid", bufs=1) as resid:
            # small constants merged into one tile: identity | mask | battn | bproj
            combo = const.tile([128, 267], f32, tag="combo")
            ident = combo[:, 0:128]
            mask = combo[:, 128:256]
            battn_sb = combo[:, 256:259]
            bp_sb = combo[:, 259:267]
            make_identity(nc, ident)
            # mask[k, q] = 1.0 where k <= q else 0 (upper triangular incl diag)
            nc.gpsimd.memset(mask, 0.0)
            nc.gpsimd.affine_select(
                out=mask, in_=mask,
                compare_op=mybir.AluOpType.is_gt,
                fill=1.0, base=0, pattern=[[-1, 128]], channel_multiplier=1,
            )
            nc.sync.dma_start(battn_sb, battn[:])
            nc.sync.dma_start(bp_sb, bproj[:])
            wp_sb = const.tile([128, C], f32r, tag="wp")
            nc.sync.dma_start(wp_sb[:], wproj[:])

            # resident activations (f32r, matmul operands all at base partition 0)
            kT = resid.tile([128, NCH, 512], f32r, tag="kT")
            qz0 = resid.tile([128, NCH, 512], f32r, tag="qz0")
            qz1 = resid.tile([128, NCH, 512], f32r, tag="qz1")
            vprf = resid.tile([128, 8576], f32r, tag="vpr")
            vpr = vprf[:, 0:8448].rearrange("p (kb c) -> p kb c", c=132)
            ones1 = vprf[0:1, 8448:8576]
            nc.gpsimd.memset(ones1.bitcast(f32), 1.0)

            nc.vector.memset(qz0[64:128, :, :].bitcast(f32), 0.0)
            nc.gpsimd.memset(qz1[0:64, :, :].bitcast(f32), 0.0)
            nc.gpsimd.memset(vpr[:, :, 64:66].bitcast(f32), 1.0)
            nc.gpsimd.memset(vpr[:, :, 130:132].bitcast(f32), 1.0)

            # ---------------- Phase A: QKV + V transposes ----------------
            with tc.tile_pool(name="phA", bufs=1) as phA, \
                 tc.tile_pool(name="xin", bufs=4) as xin, \
                 tc.tile_pool(name="psA", bufs=2, space="PSUM") as psA:
                w_sb = phA.tile([128, 8, 384], f32r, tag="w_sb")
                nc.sync.dma_start(w_sb[:], wqkv.rearrange("(ko p) m -> p ko m", p=128))
                xTr = xT.rearrange("(ko p) t -> p ko t", p=128)
                for ch in range(NCH):
                    # two half-K input tiles per chunk (double-buffered)
                    xa = xin.tile([128, 4, 512], f32r, tag="xc")
                    nc.sync.dma_start(xa[:], xTr[:, 0:4, ch * 512:(ch + 1) * 512])
                    xb = xin.tile([128, 4, 512], f32r, tag="xc")
                    nc.sync.dma_start(xb[:], xTr[:, 4:8, ch * 512:(ch + 1) * 512])
                    for m in range(3):
                        ps = psA.tile([128, 512], f32, tag=f"m{m}")
                        for ko in range(8):
                            xsrc = xa if ko < 4 else xb
                            nc.tensor.matmul(ps[:], w_sb[:, ko, m * 128:(m + 1) * 128],
                                             xsrc[:, ko % 4, :], start=(ko == 0), stop=(ko == 7))
                        if m == 0:
                            nc.vector.tensor_scalar_add(qz0[0:64, ch, :], ps[0:64, :],
                                                        battn_sb[0:64, 0:1])
                            nc.vector.tensor_scalar_add(qz1[64:128, ch, :], ps[64:128, :],
                                                        battn_sb[64:128, 0:1])
                        elif m == 1:
                            nc.vector.tensor_scalar_add(kT[:, ch, :], ps[:], battn_sb[:, 1:2])
                        else:
                            vs = phA.tile([128, 512], f32, tag="vs")
                            nc.vector.tensor_scalar_add(vs[:], ps[:], battn_sb[:, 2:3])
                            for t in range(4):
                                pst = psA.tile([128, 128], f32, tag="tp")
                                nc.tensor.transpose(pst[:], vs[:, t * 128:(t + 1) * 128], ident)
                                gkb = ch * 4 + t
                                nc.vector.tensor_copy(vpr[:, gkb, 0:64], pst[:, 0:64])
                                nc.vector.tensor_copy(vpr[:, gkb, 66:130], pst[:, 64:128])

            # ---------------- Phase B: attention + inline projection ----------------
            with tc.tile_pool(name="pp", bufs=3) as ppool, \
                 tc.tile_pool(name="nrm", bufs=1) as nrm, \
                 tc.tile_pool(name="sby", bufs=1) as sbyp, \
                 tc.tile_pool(name="ytc", bufs=3) as ytc, \
                 tc.tile_pool(name="ob", bufs=2) as obp, \
                 tc.tile_pool(name="psB", bufs=2, space="PSUM") as psB:
                def emit_proj(qch, yTch):
                    # deferred projection: runs on the PE while the *next*
                    # chunk's normalize chain is still in flight
                    for od in range(8):
                        psP = psB.tile([128, 512], f32, tag="s")
                        nc.tensor.matmul(psP[:], wp_sb[:, od * 128:(od + 1) * 128],
                                         yTch[:], start=True, stop=True)
                        oSb = obp.tile([128, 512], f32, tag="o")
                        if od % 2 == 0:
                            nc.vector.tensor_scalar_add(oSb[:], psP[:], bp_sb[:, od:od + 1])
                        else:
                            nc.scalar.activation(oSb[:], psP[:],
                                                 mybir.ActivationFunctionType.Identity,
                                                 bias=bp_sb[:, od:od + 1])
                        nc.sync.dma_start(
                            outT[od * 128:(od + 1) * 128, qch * 512:(qch + 1) * 512],
                            oSb[:])

                pending = []
                for b in range(B):
                    for j in range(CHB):
                        qch = b * CHB + j
                        psY = psB.tile([128, 2, 512], f32, tag="y")
                        nkb = 4 * j + 4

                        def emit_S(kb):
                            vstart = max(0, kb * 128 - j * 512)
                            kch = b * CHB + kb // 4
                            ksub = (kb % 4) * 128
                            psS = psB.tile([128, 2, 512], f32, tag="s")
                            nc.tensor.matmul(psS[:, 0, vstart:], kT[:, kch, ksub:ksub + 128],
                                             qz0[:, qch, vstart:], start=True, stop=True)
                            nc.tensor.matmul(psS[:, 1, vstart:], kT[:, kch, ksub:ksub + 128],
                                             qz1[:, qch, vstart:], start=True, stop=True)
                            return psS, vstart

                        # software-pipelined: S(kb+1) is emitted ahead of PV(kb)
                        # so the in-order PE never waits on exp(kb)
                        prev = emit_S(0)
                        for kb in range(nkb):
                            psS, vstart = prev
                            Pb = ppool.tile([128, 2, 512], f32r, tag="p")
                            nc.scalar.activation(Pb[:, :, vstart:], psS[:, :, vstart:],
                                                 mybir.ActivationFunctionType.Exp, scale=SCALE)
                            if kb + 1 < nkb:
                                prev = emit_S(kb + 1)
                            if kb >= 4 * j:
                                nc.vector.tensor_mul(
                                    Pb[:, :, vstart:vstart + 128],
                                    Pb[:, :, vstart:vstart + 128],
                                    mask[:, None, :].to_broadcast((128, 2, 128)))
                            gkb = b * NKB_B + kb
                            nc.tensor.matmul(psY[0:66, 0, vstart:], vpr[:, gkb, 0:66],
                                             Pb[:, 0, vstart:], start=(kb == 0), stop=(kb == nkb - 1))
                            nc.tensor.matmul(psY[0:66, 1, vstart:], vpr[:, gkb, 66:132],
                                             Pb[:, 1, vstart:], start=(kb == 0), stop=(kb == nkb - 1))
                        # previous chunk's projection first: its inputs are
                        # ready, so the PE stays busy while this chunk's
                        # normalize chain (DVE/gpsimd/DMA) runs
                        if len(pending) >= 2:
                            emit_proj(*pending.pop(0))
                        # normalize: rows 0..63 = yT raw, row 64 = denom (65 dup)
                        sbY = sbyp.tile([66, 2, 512], f32, tag="sby")
                        nc.vector.tensor_copy(sbY[:, :, :], psY[0:66, :, :])
                        sc = nrm.tile([128, 16], f32, tag="sc")
                        nc.sync.dma_start(sc[:, 0:8], sbY[64:65, :, :])
                        nc.vector.reciprocal(sc[:, 8:16], sc[:, 0:8])
                        rr = nrm.tile([1, 1024], f32r, tag="rr")
                        nc.sync.dma_start(rr[0:1, :], sc[:, 8:16].bitcast(f32r))
                        psRB = psB.tile([128, 2, 512], f32, tag="y")
                        nc.tensor.matmul(psRB[:, 0, :], ones1[:, 0:128], rr[0:1, 0:512],
                                         start=True, stop=True)
                        nc.tensor.matmul(psRB[:, 1, :], ones1[:, 0:128], rr[0:1, 512:1024],
                                         start=True, stop=True)
                        yTch = ytc.tile([128, 512], f32r, tag="yt")
                        nc.vector.tensor_mul(yTch[0:64, :], sbY[0:64, 0, :], psRB[0:64, 0, :])
                        yst = nrm.tile([64, 512], f32r, tag="yst")
                        nc.vector.tensor_mul(yst[:], sbY[0:64, 1, :], psRB[0:64, 1, :])
                        nc.sync.dma_start(yTch[64:128, :], yst[:])
                        pending.append((qch, yTch))
                for pq in pending:
                    emit_proj(*pq)

    nc.compile()
    return nc


def _get_program():
    if "nc" not in _CACHE:
        _CACHE["nc"] = _build_program()
    return _CACHE["nc"]


def kernel(x, w_attn, b_attn, w_proj, b_proj, _trace=False):
    from concourse.bass_utils import run_bass_kernel_spmd

    nc = _get_program()

    x = np.asarray(x, dtype=np.float32)
    w_attn = np.asarray(w_attn, dtype=np.float32)
    b_attn = np.asarray(b_attn, dtype=np.float32)
    w_proj = np.asarray(w_proj, dtype=np.float32)
    b_proj = np.asarray(b_proj, dtype=np.float32)

    xT_np = np.ascontiguousarray(x.reshape(TOK, C).T)

    in_maps = []
    for c in range(NCORES):
        lo, hi = c * 128, (c + 1) * 128
        wq = w_attn[:, lo:hi]
        wk = w_attn[:, C + lo:C + hi]
        wv = w_attn[:, 2 * C + lo:2 * C + hi]
        wqkv_np = np.ascontiguousarray(np.concatenate([wq, wk, wv], axis=1))
        bq = b_attn[lo:hi]
        bk = b_attn[C + lo:C + hi]
        bv = b_attn[2 * C + lo:2 * C + hi]
        battn_np = np.ascontiguousarray(np.stack([bq, bk, bv], axis=1))  # [128, 3]
        wproj_np = np.ascontiguousarray(w_proj[lo:hi, :])
        if c == 0:
            bproj_np = np.ascontiguousarray(b_proj.reshape(8, 128).T)
        else:
            bproj_np = np.zeros((128, 8), dtype=np.float32)
        in_maps.append({
            "xT": xT_np,
            "wqkv": wqkv_np,
            "battn": battn_np,
            "wproj": wproj_np,
            "bproj": bproj_np,
        })

    res = run_bass_kernel_spmd(nc, in_maps, core_ids=list(range(NCORES)),
                               trace=_trace)
    acc = res.results[0]["outT"].astype(np.float32).copy()
    for c in range(1, NCORES):
        acc += res.results[c]["outT"]
    out = np.ascontiguousarray(acc.T).reshape(B, T, C)
    if _trace:
        kernel.last_exec_time_ns = res.exec_time_ns
        kernel.last_scope_times = res.per_core_scope_times
        kernel.last_trace = res.instructions_and_trace
    return out

